# revision 41
# baseline (speedup 1.0000x reference)
"""Bass/Trainium2 SPMD kernel for a 2-layer GCN encoder.

Math (per reference):
    src/dst = edges + self-loops
    deg[v]  = #edges with dst==v (incl self-loop);  dinv = 1/sqrt(deg)
    layer(x, W, b): out[d] = dinv[d] * sum_{e: dst_e==d} dinv[src_e] * (x@W)[src_e] + b
    y = layer1(sigmoid(layer0(x, W0, b0)), W1, b1)

Distribution: nodes sharded contiguously across 8 cores (6250 each); edges
owned by the destination core.  All feature math in bf16 (f32 PSUM).

The per-node tables (h1 = dinv*(x@W0), then t2 = dinv*sigmoid(...)) are
AllGathered in two chunks split at local index 4096 (block-aligned):
chunk A is ready earlier, so each layer's aggregation runs in two passes:

  pass 1: gather src rows from table A, accumulate each dst block's
          partial sum in PSUM, park it in SBUF (bf16).
  pass 2: gather from table B (its AllGather overlapped pass 1),
          re-inject the partial via an identity matmul, add the bias
          (rank-1 sqdeg x b), the appended self-loop (identity matmul of
          the local block), and close.

Layer 1 closes with t2 = dinv*sigmoid(dinv*agg) per block (normal
[dst, f] orientation).  Layer 2 runs with matmul roles swapped so the
accumulator is aggT[f, dst]; closing does aggT->SBUF, then a per-block
GEMM with W1 + rank-1 bias, dinv applied on the PSUM->SBUF copy. This
keeps every matmul bf16 (no fp32 PE penalty) and needs no transposes.

dma_gather descriptor generation on the Q7 pairs is the critical
resource (~5-6ns per gathered row, engine-serial): gathers cycle over 4
SWDGE queues for ~1.7x generation overlap, a dummy collective absorbs
the one-time CC-init barrier under the load phase, and the two-pass
schedule keeps gathers ahead of every AllGather wait in the in-order
gpsimd queue.
"""

import math

import numpy as np
import ml_dtypes

import concourse.bacc as bacc
import concourse.bass as bass
import concourse.mybir as mybir
import concourse.tile as tile
from concourse.bass_utils import run_bass_kernel_spmd

P = 128
F32 = mybir.dt.float32
BF16 = mybir.dt.bfloat16
I16 = mybir.dt.int16

N_NODES = 50000
N_CORES = 8
F0, F1, F2 = 128, 128, 64
GROUP_BLOCKS = 1  # dst blocks per dma_gather batch
HALF_A = 4096  # local-index split for the table halves (block-aligned)
# Per-(block,half) edge-segment alignment. Must stay 128: sub-128 matmul
# pieces with different base partitions back-to-back hard-crash the PE.
SEG_ALIGN = 128
BF = ml_dtypes.bfloat16


class Plan:
    """Compile-time schedule, identical across cores (SPMD)."""

    def __init__(self, n_nodes, n_cores, gb):
        assert n_nodes % n_cores == 0
        self.n_nodes = n_nodes
        self.n_cores = n_cores
        self.npc = n_nodes // n_cores
        self.nblk = math.ceil(self.npc / P)
        self.ha = HALF_A
        self.hb = self.npc - HALF_A
        assert self.ha * n_cores <= 32768 and self.ha % P == 0
        self.blk_a = self.ha // P  # blocks fully in half A
        self.gb = gb
        self.groups = [
            list(range(i, min(i + gb, self.nblk))) for i in range(0, self.nblk, gb)
        ]
        self.g_of = {}
        for gi, blocks in enumerate(self.groups):
            for b in blocks:
                self.g_of[b] = gi
        self.SZ = None  # [nblk, 2] int, multiples of SEG_ALIGN
        self.seg_off = {}  # (b, h) -> edge offset within its gather
        self.seg_idx16 = {}  # (g_idx, h) -> int16-column base of that gather
        self.seg_colbase = {}  # (g_idx, h) -> global chunk-column base
        self.gather_nid = {}  # (g_idx, h) -> num idxs
        self.ncols = 0
        self.tot16 = 0

    def finalize(self, sz):
        self.SZ = sz
        col = 0
        i16 = 0
        for gi, blocks in enumerate(self.groups):
            for h in (0, 1):
                off = 0
                for b in blocks:
                    self.seg_off[(b, h)] = off
                    off += int(self.SZ[b, h])
                nid = off
                self.gather_nid[(gi, h)] = nid
                self.seg_idx16[(gi, h)] = i16
                self.seg_colbase[(gi, h)] = col
                col += (nid + P - 1) // P
                i16 += nid // 16
        self.ncols = col
        self.tot16 = i16


def _build_metadata(edges, n_nodes, n_cores, gb=GROUP_BLOCKS):
    """Host-side integer preprocessing: shard + sort edges, build gather
    indices / slot vectors / degree tables.  Returns (plan, per_core dict).

    The appended self-loop edges are NOT in the gather lists (added on-chip
    via identity matmuls); natural src==dst edges stay."""
    plan = Plan(n_nodes, n_cores, gb)
    npc, nblk, ha = plan.npc, plan.nblk, plan.ha

    src = np.asarray(edges[0], dtype=np.int64)
    dst = np.asarray(edges[1], dtype=np.int64)
    # deg includes the appended self-loops
    deg = (np.bincount(dst, minlength=n_nodes) + 1).astype(np.float32)

    owner = dst // npc
    ldst = dst % npc
    blk = ldst // P
    slot = (ldst % P).astype(np.float32)
    csrc = src // npc
    lsrc = src % npc
    hh = (lsrc >= ha).astype(np.int64)
    idxval = np.where(hh == 0, csrc * ha + lsrc, csrc * plan.hb + (lsrc - ha))
    cell = ((owner * nblk) + blk) * 2 + hh
    order = np.lexsort((idxval, cell))
    cell_s = cell[order]
    idx_s = idxval[order]
    slot_s = slot[order]

    ncells = n_cores * nblk * 2
    counts = np.bincount(cell_s, minlength=ncells).reshape(n_cores, nblk, 2)
    starts = np.concatenate([[0], np.cumsum(counts.reshape(-1))])[:-1].reshape(
        n_cores, nblk, 2
    )
    sz = np.maximum(counts.max(axis=0), 0)
    sz = (np.ceil(sz / SEG_ALIGN).astype(np.int64)) * SEG_ALIGN  # [nblk, 2]
    plan.finalize(sz)

    ncols = plan.ncols
    tot16 = plan.tot16

    per_core = []
    for c in range(n_cores):
        idx16 = np.zeros((16, tot16), np.int16)
        slots_t = np.full((P, ncols), -1.0, np.float32)
        for gi, blocks in enumerate(plan.groups):
            for h in (0, 1):
                i16b = plan.seg_idx16[(gi, h)] * 16
                colb = plan.seg_colbase[(gi, h)] * P
                for b in blocks:
                    n = int(counts[c, b, h])
                    s0 = int(starts[c, b, h])
                    if n:
                        j = plan.seg_off[(b, h)] + np.arange(n)
                        seg_src = idx_s[s0 : s0 + n].astype(np.int16)
                        ji = i16b + j
                        idx16[ji % 16, ji // 16] = seg_src
                        jc = colb + j
                        slots_t[jc % P, jc // P] = slot_s[s0 : s0 + n]
        deg_loc = np.ones(nblk * P, np.float32)
        deg_loc[:npc] = deg[c * npc : (c + 1) * npc]
        deg_t = deg_loc.reshape(nblk, P).T.copy()  # [P, nblk]
        per_core.append(
            dict(
                idx16=np.tile(idx16, (8, 1)),  # [128, tot16]
                slots=slots_t,
                degt=deg_t,
                sqrow=np.sqrt(deg_loc).reshape(1, -1).copy(),
            )
        )
    return plan, per_core


def _build_nc(plan, f0, f1, f2):
    """Build the SPMD bass program (same for every core)."""
    npc, nblk = plan.npc, plan.nblk
    ha, hb, blk_a = plan.ha, plan.hb, plan.blk_a
    nc = bacc.Bacc(
        "TRN2",
        target_bir_lowering=False,
        debug=False,
        num_devices=plan.n_cores,
        num_swdge_queues=4,
    )

    # I/O
    xT_d = nc.dram_tensor("xT", [f0, npc], BF16, kind="ExternalInput")
    w0_d = nc.dram_tensor("W0", [f0, f1], BF16, kind="ExternalInput")
    w1_d = nc.dram_tensor("W1", [f1, f2], BF16, kind="ExternalInput")
    b0_d = nc.dram_tensor("b0", [1, f1], F32, kind="ExternalInput")
    b1_d = nc.dram_tensor("b1", [1, f2], F32, kind="ExternalInput")
    iota_d = nc.dram_tensor("iota", [P, P], F32, kind="ExternalInput")
    ident_d = nc.dram_tensor("ident", [P, P], F32, kind="ExternalInput")
    identb_d = nc.dram_tensor("identb", [P, P], BF16, kind="ExternalInput")
    degt_d = nc.dram_tensor("degt", [P, nblk], F32, kind="ExternalInput")
    sqrow_d = nc.dram_tensor("sqrow", [1, nblk * P], F32, kind="ExternalInput")
    idx_d = nc.dram_tensor("idx16", [P, plan.tot16], I16, kind="ExternalInput")
    widx_d = nc.dram_tensor("widx", [P, 8], I16, kind="ExternalInput")
    slots_d = nc.dram_tensor("slots", [P, plan.ncols], F32, kind="ExternalInput")
    y_d = nc.dram_tensor("y", [npc, f2], F32, kind="ExternalOutput")

    rg = [list(range(plan.n_cores))]
    AF = mybir.ActivationFunctionType

    with tile.TileContext(nc) as tc:
        with (
            tc.tile_pool(name="dram", bufs=1, space="DRAM") as dramp,
            tc.tile_pool(name="const", bufs=1) as constp,
            tc.tile_pool(name="gath", bufs=8) as gpool,
            tc.tile_pool(name="sel", bufs=4) as spool,
            tc.tile_pool(name="stage", bufs=6) as stpool,
            tc.tile_pool(name="pgemm", bufs=2, space="PSUM") as pgemm,
            tc.tile_pool(name="pscat", bufs=4, space="PSUM") as pscat,
        ):
            # per-layer DRAM: local halves (split by local node index at ha)
            # + gathered full half-tables (AllGather outputs)
            locs = []
            tabs = []
            for L in (1, 2):
                la = dramp.tile([ha, f1], BF16, name=f"loc{L}a")
                lb = dramp.tile([hb, f1], BF16, name=f"loc{L}b")
                ta = dramp.tile(
                    [ha * plan.n_cores, f1], BF16, addr_space="Shared",
                    name=f"tab{L}a",
                )
                tb = dramp.tile(
                    [hb * plan.n_cores, f1], BF16, addr_space="Shared",
                    name=f"tab{L}b",
                )
                locs.append((la, lb))
                tabs.append((ta, tb))

            def allgather(loc, tab):
                nc.gpsimd.collective_compute(
                    "AllGather",
                    mybir.AluOpType.bypass,
                    replica_groups=rg,
                    ins=[loc[:, :].opt()],
                    outs=[tab[:, :].opt()],
                )

            def load_const(name, dram, shape, dtype=F32):
                t = constp.tile(shape, dtype, name=name)
                nc.sync.dma_start(out=t[:], in_=dram[:])
                return t

            xT_t = constp.tile([f0, npc], BF16, name="xT_t")
            for xc in range(4):
                c0 = (npc * xc // 4) // P * P
                c1 = npc if xc == 3 else (npc * (xc + 1) // 4) // P * P
                nc.sync.dma_start(out=xT_t[:, c0:c1], in_=xT_d[:, c0:c1])
            w0_t = load_const("w0_t", w0_d, [f0, f1], BF16)
            degt_t = load_const("degt_t", degt_d, [P, nblk])
            widx_t = load_const("widx_t", widx_d, [P, 8], I16)

            # dinv = 1/sqrt(deg)
            sq_t = constp.tile([P, nblk], F32, name="sq_t")
            nc.scalar.activation(sq_t[:], degt_t[:], AF.Sqrt)
            dinv_t = constp.tile([P, nblk], F32, name="dinv_t")
            nc.vector.reciprocal(dinv_t[:], sq_t[:])

            # local h rows kept in SBUF for the self-loop matmuls, and
            # per-block pass-1 partials parked between the two passes
            hs_all = constp.tile([P, nblk, f1], BF16, name="hs_all")
            t2_all = constp.tile([P, nblk, f1], BF16, name="t2_all")
            part_all = constp.tile([P, nblk, f1], BF16, name="part_all")

            # dummy collective issued first: absorbs the one-time CC init
            # barrier (~30-120us) under the constant-load/GEMM head
            cc_in = dramp.tile([1, P], F32, name="cc_in")
            cc_out = dramp.tile([plan.n_cores, P], F32, addr_space="Shared",
                                name="cc_out")
            allgather(cc_in, cc_out)

            # warm the Q7 dma_gather ucode on every queue pair (first gather
            # per pair pays ~29us icache fill; do it under the GEMM head)
            for q in range(4):
                warm_t = constp.tile([P, 1, 64], F32, name=f"warm_t{q}")
                nc.gpsimd.dma_gather(
                    warm_t[:],
                    ident_d[:, 0:64],
                    widx_t[:, 0:8],
                    128,
                    128,
                    64,
                    elem_step=P,
                    single_packet=False,
                    queue_num=q,
                )

            def row_dma(loc_a, loc_b, src_tile, b, wt):
                """DMA rows [b*P, b*P+wt) of the local table to the A/B
                halves (split at local index ha; ha is block-aligned)."""
                r0 = b * P
                if r0 < ha:
                    nc.sync.dma_start(
                        out=loc_a[r0 : r0 + wt, :], in_=src_tile[:wt, :]
                    )
                else:
                    nc.sync.dma_start(
                        out=loc_b[r0 - ha : r0 - ha + wt, :], in_=src_tile[:wt, :]
                    )

            # ---- layer-1 GEMM: hs = dinv * (x @ W0), kept in SBUF + DRAM ----
            # zero the padded tail of the last block first: its rows feed the
            # self-loop identity matmul, and NaN garbage * 0 = NaN on the PE
            nc.vector.memset(hs_all[:, nblk - 1, :], 0.0)
            for t in range(nblk):
                wt = min(P, npc - t * P)
                hp = pgemm.tile([P, f1], F32, name="hp")
                nc.tensor.matmul(
                    hp[:wt, :],
                    xT_t[:, t * P : t * P + wt],
                    w0_t[:],
                    start=True,
                    stop=True,
                )
                nc.scalar.activation(
                    hs_all[:wt, t, :],
                    hp[:wt, :],
                    AF.Copy,
                    scale=dinv_t[:wt, t : t + 1],
                )
                row_dma(locs[0][0], locs[0][1], hs_all[:, t, :], t, wt)
                if t == blk_a - 1:
                    allgather(locs[0][0], tabs[0][0])
            allgather(locs[0][1], tabs[0][1])

            # big constant loads deferred here so the GEMM row-DMAs (which
            # gate AG-L1A) are not queued behind them on the sync DMA queue;
            # these finish during the AllGathers, before the first consumers
            idx_t = load_const("idx_t", idx_d, [P, plan.tot16], I16)
            slots_t = load_const("slots_t", slots_d, [P, plan.ncols])
            w1_t = load_const("w1_t", w1_d, [f1, f2], BF16)
            b0_t = load_const("b0_t", b0_d, [1, f1])
            b1_t = load_const("b1_t", b1_d, [1, f2])
            iota_t = load_const("iota_t", iota_d, [P, P])
            identb_t = load_const("identb_t", identb_d, [P, P], BF16)
            sqrow_t = load_const("sqrow_t", sqrow_d, [1, nblk * P])

            qc = [0]  # global gather-queue cycle

            MAXC = 16  # max 128-row columns per dma_gather (smaller gathers
            # pipeline better through the gpsimd dispatch overlap)

            def gather_half(layer, gi, h):
                nid = plan.gather_nid[(gi, h)]
                if nid == 0:
                    return None
                ncol = (nid + P - 1) // P
                i0 = plan.seg_idx16[(gi, h)]
                tiles = []
                for j in range(0, ncol, MAXC):
                    nch = min(MAXC, ncol - j)
                    nid_j = nch * P
                    g_tile = gpool.tile(
                        [P, nch, f1], BF16, tag="gath", name=f"g{layer}{gi}_{h}_{j}"
                    )
                    ij = i0 + j * P // 16
                    nc.gpsimd.dma_gather(
                        g_tile[:],
                        tabs[layer - 1][h][:, :],
                        idx_t[:, ij : ij + nid_j // 16],
                        nid_j,
                        nid_j,
                        f1,
                        single_packet=False,
                        queue_num=qc[0] % 4,
                    )
                    qc[0] += 1
                    tiles.append(g_tile)
                return tiles

            def block_pieces(b, h):
                sz = int(plan.SZ[b, h])
                if sz == 0:
                    return None
                off = plan.seg_off[(b, h)]
                return (off // P, sz // P)  # (first col, ncols)

            def build_sel_group(gi, h):
                nid = plan.gather_nid[(gi, h)]
                if nid == 0:
                    return None
                nch = (nid + P - 1) // P
                colb = plan.seg_colbase[(gi, h)]
                sel = spool.tile([P, nch, P], BF16, tag="sel", name="sel")
                nc.vector.tensor_tensor(
                    out=sel[:],
                    in0=slots_t[:, colb : colb + nch].to_broadcast([P, nch, P]),
                    in1=iota_t[:, :]
                    .rearrange("p (a b) -> p a b", a=1)
                    .to_broadcast([P, nch, P]),
                    op=mybir.AluOpType.is_equal,
                )
                return sel

            def seg_matmuls(pb, g_tiles, sel, span, swapped, start, stop):
                c0, nch = span
                for k, c in enumerate(range(c0, c0 + nch)):
                    st = start and k == 0
                    sp = stop and k == nch - 1
                    gt = g_tiles[c // MAXC][:, c % MAXC, :]
                    if not swapped:
                        nc.tensor.matmul(
                            pb[:], sel[:, c, :], gt, start=st, stop=sp,
                        )
                    else:
                        nc.tensor.matmul(
                            pb[:], gt, sel[:, c, :], start=st, stop=sp,
                        )

            def scatter_pass1(layer):
                """Half-A gathers; park each block's partial sum in SBUF."""
                swapped = layer == 2
                for gi, blocks in enumerate(plan.groups):
                    g_tile = gather_half(layer, gi, 0)
                    sel = build_sel_group(gi, 0)
                    for b in blocks:
                        span = block_pieces(b, 0)
                        if span is None:
                            continue
                        pb = pscat.tile([P, f1], F32, name="pb")
                        seg_matmuls(
                            pb, g_tile, sel, span, swapped,
                            start=True, stop=True,
                        )
                        nc.scalar.activation(part_all[:, b, :], pb[:], AF.Copy)

            def scatter_pass2(layer):
                """Half-B gathers; bias + self-loop + partial re-injection +
                half-B pieces, then close out the block."""
                swapped = layer == 2
                # for layer 1: trigger the next layer's half-A AllGather a
                # couple of groups after block blk_a-1's t2 DMA is issued, so
                # the collective's wait never stalls pending gathers
                ag_group = plan.g_of[blk_a - 1] + 1 if layer == 1 else None
                for gi, blocks in enumerate(plan.groups):
                    g_tile = gather_half(layer, gi, 1)
                    sel = build_sel_group(gi, 1)
                    if gi == ag_group:
                        allgather(locs[1][0], tabs[1][0])
                    for b in blocks:
                        wb = min(P, npc - b * P)
                        span = block_pieces(b, 1)
                        pb = pscat.tile([P, f1], F32, name="pb")
                        if not swapped:
                            nc.tensor.matmul(
                                pb[:],
                                sqrow_t[0:1, b * P : (b + 1) * P],
                                b0_t[:],
                                start=True,
                                stop=False,
                            )
                            nc.tensor.matmul(
                                pb[:], identb_t[:], hs_all[:, b, :],
                                start=False, stop=False,
                            )
                        else:
                            nc.tensor.matmul(
                                pb[:], t2_all[:, b, :], identb_t[:],
                                start=True, stop=False,
                            )
                        if block_pieces(b, 0) is not None:
                            # re-inject the pass-1 partial
                            nc.tensor.matmul(
                                pb[:], identb_t[:], part_all[:, b, :],
                                start=False, stop=span is None,
                            )
                        if span is not None:
                            seg_matmuls(
                                pb, g_tile, sel, span, swapped,
                                start=False, stop=True,
                            )
                        if not swapped:
                            # t2 = dinv * sigmoid(dinv * agg)
                            ob = stpool.tile([P, f1], F32, tag="ob", name="ob")
                            nc.scalar.activation(
                                ob[:], pb[:], AF.Sigmoid,
                                scale=dinv_t[:, b : b + 1],
                            )
                            nc.scalar.activation(
                                t2_all[:, b, :], ob[:], AF.Copy,
                                scale=dinv_t[:, b : b + 1],
                            )
                            row_dma(locs[1][0], locs[1][1], t2_all[:, b, :], b, wb)
                        else:
                            aggT = stpool.tile(
                                [P, f1], BF16, tag="aggT", name="aggT"
                            )
                            nc.scalar.activation(aggT[:], pb[:], AF.Copy)
                            yp = pgemm.tile([P, f2], F32, name="yp")
                            nc.tensor.matmul(
                                yp[:wb, :], aggT[:, :wb], w1_t[:],
                                start=True, stop=False,
                            )
                            nc.tensor.matmul(
                                yp[:wb, :],
                                sqrow_t[0:1, b * P : b * P + wb],
                                b1_t[:],
                                start=False,
                                stop=True,
                            )
                            yo = stpool.tile([P, f2], F32, tag="yo", name="yo")
                            nc.scalar.activation(
                                yo[:wb, :], yp[:wb, :], AF.Copy,
                                scale=dinv_t[:wb, b : b + 1],
                            )
                            nc.sync.dma_start(
                                out=y_d[b * P : b * P + wb, :], in_=yo[:wb, :]
                            )

            scatter_pass1(1)
            scatter_pass2(1)
            scatter_pass1(2)
            allgather(locs[1][1], tabs[1][1])
            scatter_pass2(2)

    nc.compile()
    return nc


def _make_in_maps(x, W0, b0, W1, b1, plan, per_core):
    npc = plan.npc
    x = np.asarray(x, dtype=np.float32)
    shared = dict(
        W0=np.asarray(W0, np.float32).astype(BF),
        W1=np.asarray(W1, np.float32).astype(BF),
        b0=np.asarray(b0, np.float32).reshape(1, -1),
        b1=np.asarray(b1, np.float32).reshape(1, -1),
        iota=np.tile(np.arange(P, dtype=np.float32)[None, :], (P, 1)).copy(),
        ident=np.eye(P, dtype=np.float32),
        identb=np.eye(P, dtype=np.float32).astype(BF),
    )
    in_maps = []
    for c in range(plan.n_cores):
        m = dict(shared)
        m["xT"] = np.ascontiguousarray(x[c * npc : (c + 1) * npc, :].T).astype(BF)
        m["idx16"] = per_core[c]["idx16"]
        m["widx"] = np.zeros((P, 8), np.int16)
        m["slots"] = per_core[c]["slots"]
        m["degt"] = per_core[c]["degt"]
        m["sqrow"] = per_core[c]["sqrow"]
        in_maps.append(m)
    return in_maps


_CACHE = {}


def build(x, edges, W0, b0, W1, b1, n_nodes=N_NODES, n_cores=N_CORES,
          gb=GROUP_BLOCKS):
    """Returns (nc, in_maps, plan). Cached on the edge structure size."""
    plan, per_core = _build_metadata(edges, n_nodes, n_cores, gb)
    key = (n_nodes, n_cores, gb, tuple(plan.SZ.reshape(-1).tolist()))
    if key not in _CACHE:
        _CACHE[key] = _build_nc(plan, x.shape[1], W0.shape[1], W1.shape[1])
    nc = _CACHE[key]
    in_maps = _make_in_maps(x, W0, b0, W1, b1, plan, per_core)
    return nc, in_maps, plan


def kernel(x, edges, W0, b0, W1, b1):
    x = np.asarray(x)
    nc, in_maps, plan = build(x, edges, W0, b0, W1, b1)
    res = run_bass_kernel_spmd(nc, in_maps, list(range(plan.n_cores)))
    y = np.concatenate([r["y"] for r in res.results], axis=0)
    return y.astype(np.float32)


# revision 43
# speedup vs baseline: 1.0041x; 1.0041x over previous
"""Bass/Trainium2 SPMD kernel for a 2-layer GCN encoder.

Math (per reference):
    src/dst = edges + self-loops
    deg[v]  = #edges with dst==v (incl self-loop);  dinv = 1/sqrt(deg)
    layer(x, W, b): out[d] = dinv[d] * sum_{e: dst_e==d} dinv[src_e] * (x@W)[src_e] + b
    y = layer1(sigmoid(layer0(x, W0, b0)), W1, b1)

Distribution: nodes sharded contiguously across 8 cores (6250 each); edges
owned by the destination core.  All feature math in bf16 (f32 PSUM).

The per-node tables (h1 = dinv*(x@W0), then t2 = dinv*sigmoid(...)) are
AllGathered in two chunks split at local index 4096 (block-aligned):
chunk A is ready earlier, so each layer's aggregation runs in two passes:

  pass 1: gather src rows from table A, accumulate each dst block's
          partial sum in PSUM, park it in SBUF (bf16).
  pass 2: gather from table B (its AllGather overlapped pass 1),
          re-inject the partial via an identity matmul, add the bias
          (rank-1 sqdeg x b), the appended self-loop (identity matmul of
          the local block), and close.

Layer 1 closes with t2 = dinv*sigmoid(dinv*agg) per block (normal
[dst, f] orientation).  Layer 2 runs with matmul roles swapped so the
accumulator is aggT[f, dst]; closing does aggT->SBUF, then a per-block
GEMM with W1 + rank-1 bias, dinv applied on the PSUM->SBUF copy. This
keeps every matmul bf16 (no fp32 PE penalty) and needs no transposes.

dma_gather descriptor generation on the Q7 pairs is the critical
resource (~5-6ns per gathered row, engine-serial): gathers cycle over 4
SWDGE queues for ~1.7x generation overlap, a dummy collective absorbs
the one-time CC-init barrier under the load phase, and the two-pass
schedule keeps gathers ahead of every AllGather wait in the in-order
gpsimd queue.
"""

import math

import numpy as np
import ml_dtypes

import concourse.bacc as bacc
import concourse.bass as bass
import concourse.mybir as mybir
import concourse.tile as tile
from concourse.bass_utils import run_bass_kernel_spmd

P = 128
F32 = mybir.dt.float32
BF16 = mybir.dt.bfloat16
I16 = mybir.dt.int16

N_NODES = 50000
N_CORES = 8
F0, F1, F2 = 128, 128, 64
GROUP_BLOCKS = 1  # dst blocks per dma_gather batch
HALF_A = 4096  # local-index split for the table halves (block-aligned)
# Per-(block,half) edge-segment alignment. Must stay 128: sub-128 matmul
# pieces with different base partitions back-to-back hard-crash the PE.
SEG_ALIGN = 128
BF = ml_dtypes.bfloat16


class Plan:
    """Compile-time schedule, identical across cores (SPMD)."""

    def __init__(self, n_nodes, n_cores, gb):
        assert n_nodes % n_cores == 0
        self.n_nodes = n_nodes
        self.n_cores = n_cores
        self.npc = n_nodes // n_cores
        self.nblk = math.ceil(self.npc / P)
        self.ha = HALF_A
        self.hb = self.npc - HALF_A
        assert self.ha * n_cores <= 32768 and self.ha % P == 0
        self.blk_a = self.ha // P  # blocks fully in half A
        self.gb = gb
        self.groups = [
            list(range(i, min(i + gb, self.nblk))) for i in range(0, self.nblk, gb)
        ]
        self.g_of = {}
        for gi, blocks in enumerate(self.groups):
            for b in blocks:
                self.g_of[b] = gi
        self.SZ = None  # [nblk, 2] int, multiples of SEG_ALIGN
        self.seg_off = {}  # (b, h) -> edge offset within its gather
        self.seg_idx16 = {}  # (g_idx, h) -> int16-column base of that gather
        self.seg_colbase = {}  # (g_idx, h) -> global chunk-column base
        self.gather_nid = {}  # (g_idx, h) -> num idxs
        self.ncols = 0
        self.tot16 = 0

    def finalize(self, sz):
        self.SZ = sz
        col = 0
        i16 = 0
        for gi, blocks in enumerate(self.groups):
            for h in (0, 1):
                off = 0
                for b in blocks:
                    self.seg_off[(b, h)] = off
                    off += int(self.SZ[b, h])
                nid = off
                self.gather_nid[(gi, h)] = nid
                self.seg_idx16[(gi, h)] = i16
                self.seg_colbase[(gi, h)] = col
                col += (nid + P - 1) // P
                i16 += nid // 16
        self.ncols = col
        self.tot16 = i16


def _build_metadata(edges, n_nodes, n_cores, gb=GROUP_BLOCKS):
    """Host-side integer preprocessing: shard + sort edges, build gather
    indices / slot vectors / degree tables.  Returns (plan, per_core dict).

    The appended self-loop edges are NOT in the gather lists (added on-chip
    via identity matmuls); natural src==dst edges stay."""
    plan = Plan(n_nodes, n_cores, gb)
    npc, nblk, ha = plan.npc, plan.nblk, plan.ha

    src = np.asarray(edges[0], dtype=np.int64)
    dst = np.asarray(edges[1], dtype=np.int64)
    # deg includes the appended self-loops
    deg = (np.bincount(dst, minlength=n_nodes) + 1).astype(np.float32)

    owner = dst // npc
    ldst = dst % npc
    blk = ldst // P
    slot = (ldst % P).astype(np.float32)
    csrc = src // npc
    lsrc = src % npc
    hh = (lsrc >= ha).astype(np.int64)
    idxval = np.where(hh == 0, csrc * ha + lsrc, csrc * plan.hb + (lsrc - ha))
    cell = ((owner * nblk) + blk) * 2 + hh
    order = np.lexsort((idxval, cell))
    cell_s = cell[order]
    idx_s = idxval[order]
    slot_s = slot[order]

    ncells = n_cores * nblk * 2
    counts = np.bincount(cell_s, minlength=ncells).reshape(n_cores, nblk, 2)
    starts = np.concatenate([[0], np.cumsum(counts.reshape(-1))])[:-1].reshape(
        n_cores, nblk, 2
    )
    sz = np.maximum(counts.max(axis=0), 0)
    sz = (np.ceil(sz / SEG_ALIGN).astype(np.int64)) * SEG_ALIGN  # [nblk, 2]
    plan.finalize(sz)

    ncols = plan.ncols
    tot16 = plan.tot16

    per_core = []
    for c in range(n_cores):
        idx16 = np.zeros((16, tot16), np.int16)
        slots_t = np.full((P, ncols), -1.0, np.float32)
        for gi, blocks in enumerate(plan.groups):
            for h in (0, 1):
                i16b = plan.seg_idx16[(gi, h)] * 16
                colb = plan.seg_colbase[(gi, h)] * P
                for b in blocks:
                    n = int(counts[c, b, h])
                    s0 = int(starts[c, b, h])
                    if n:
                        j = plan.seg_off[(b, h)] + np.arange(n)
                        seg_src = idx_s[s0 : s0 + n].astype(np.int16)
                        ji = i16b + j
                        idx16[ji % 16, ji // 16] = seg_src
                        jc = colb + j
                        slots_t[jc % P, jc // P] = slot_s[s0 : s0 + n]
        deg_loc = np.ones(nblk * P, np.float32)
        deg_loc[:npc] = deg[c * npc : (c + 1) * npc]
        deg_t = deg_loc.reshape(nblk, P).T.copy()  # [P, nblk]
        per_core.append(
            dict(
                idx16=np.tile(idx16, (8, 1)),  # [128, tot16]
                slots=slots_t,
                degt=deg_t,
                sqrow=np.sqrt(deg_loc).reshape(1, -1).copy(),
            )
        )
    return plan, per_core


def _build_nc(plan, f0, f1, f2):
    """Build the SPMD bass program (same for every core)."""
    npc, nblk = plan.npc, plan.nblk
    ha, hb, blk_a = plan.ha, plan.hb, plan.blk_a
    nc = bacc.Bacc(
        "TRN2",
        target_bir_lowering=False,
        debug=False,
        num_devices=plan.n_cores,
        num_swdge_queues=4,
    )

    # I/O
    xT_d = nc.dram_tensor("xT", [f0, npc], BF16, kind="ExternalInput")
    w0_d = nc.dram_tensor("W0", [f0, f1], BF16, kind="ExternalInput")
    w1_d = nc.dram_tensor("W1", [f1, f2], BF16, kind="ExternalInput")
    b0_d = nc.dram_tensor("b0", [1, f1], F32, kind="ExternalInput")
    b1_d = nc.dram_tensor("b1", [1, f2], F32, kind="ExternalInput")
    iota_d = nc.dram_tensor("iota", [P, P], F32, kind="ExternalInput")
    ident_d = nc.dram_tensor("ident", [P, P], F32, kind="ExternalInput")
    identb_d = nc.dram_tensor("identb", [P, P], BF16, kind="ExternalInput")
    degt_d = nc.dram_tensor("degt", [P, nblk], F32, kind="ExternalInput")
    sqrow_d = nc.dram_tensor("sqrow", [1, nblk * P], F32, kind="ExternalInput")
    idx_d = nc.dram_tensor("idx16", [P, plan.tot16], I16, kind="ExternalInput")
    widx_d = nc.dram_tensor("widx", [P, 8], I16, kind="ExternalInput")
    slots_d = nc.dram_tensor("slots", [P, plan.ncols], F32, kind="ExternalInput")
    y_d = nc.dram_tensor("y", [npc, f2], F32, kind="ExternalOutput")

    rg = [list(range(plan.n_cores))]
    AF = mybir.ActivationFunctionType

    with tile.TileContext(nc) as tc:
        with (
            tc.tile_pool(name="dram", bufs=1, space="DRAM") as dramp,
            tc.tile_pool(name="const", bufs=1) as constp,
            tc.tile_pool(name="gath", bufs=8) as gpool,
            tc.tile_pool(name="sel", bufs=4) as spool,
            tc.tile_pool(name="stage", bufs=6) as stpool,
            tc.tile_pool(name="pgemm", bufs=2, space="PSUM") as pgemm,
            tc.tile_pool(name="pscat", bufs=4, space="PSUM") as pscat,
        ):
            # per-layer DRAM: local halves (split by local node index at ha)
            # + gathered full half-tables (AllGather outputs)
            locs = []
            tabs = []
            for L in (1, 2):
                la = dramp.tile([ha, f1], BF16, name=f"loc{L}a")
                lb = dramp.tile([hb, f1], BF16, name=f"loc{L}b")
                ta = dramp.tile(
                    [ha * plan.n_cores, f1], BF16, addr_space="Shared",
                    name=f"tab{L}a",
                )
                tb = dramp.tile(
                    [hb * plan.n_cores, f1], BF16, addr_space="Shared",
                    name=f"tab{L}b",
                )
                locs.append((la, lb))
                tabs.append((ta, tb))

            def allgather(loc, tab):
                nc.gpsimd.collective_compute(
                    "AllGather",
                    mybir.AluOpType.bypass,
                    replica_groups=rg,
                    ins=[loc[:, :].opt()],
                    outs=[tab[:, :].opt()],
                )

            def load_const(name, dram, shape, dtype=F32):
                t = constp.tile(shape, dtype, name=name)
                nc.sync.dma_start(out=t[:], in_=dram[:])
                return t

            xT_t = constp.tile([f0, npc], BF16, name="xT_t")
            for xc in range(4):
                c0 = (npc * xc // 4) // P * P
                c1 = npc if xc == 3 else (npc * (xc + 1) // 4) // P * P
                nc.sync.dma_start(out=xT_t[:, c0:c1], in_=xT_d[:, c0:c1])
            w0_t = load_const("w0_t", w0_d, [f0, f1], BF16)
            degt_t = load_const("degt_t", degt_d, [P, nblk])
            widx_t = load_const("widx_t", widx_d, [P, 8], I16)

            # dinv = 1/sqrt(deg)
            sq_t = constp.tile([P, nblk], F32, name="sq_t")
            nc.scalar.activation(sq_t[:], degt_t[:], AF.Sqrt)
            dinv_t = constp.tile([P, nblk], F32, name="dinv_t")
            nc.vector.reciprocal(dinv_t[:], sq_t[:])

            # local h rows kept in SBUF for the self-loop matmuls, and
            # per-block pass-1 partials parked between the two passes
            hs_all = constp.tile([P, nblk, f1], BF16, name="hs_all")
            t2_all = constp.tile([P, nblk, f1], BF16, name="t2_all")
            part_all = constp.tile([P, nblk, f1], BF16, name="part_all")

            # dummy collective issued first: absorbs the one-time CC init
            # barrier (~30-120us) under the constant-load/GEMM head
            cc_in = dramp.tile([1, P], F32, name="cc_in")
            cc_out = dramp.tile([plan.n_cores, P], F32, addr_space="Shared",
                                name="cc_out")
            allgather(cc_in, cc_out)

            # warm the Q7 dma_gather ucode on every queue pair (first gather
            # per pair pays ~29us icache fill; do it under the GEMM head)
            for q in range(4):
                warm_t = constp.tile([P, 1, 64], F32, name=f"warm_t{q}")
                nc.gpsimd.dma_gather(
                    warm_t[:],
                    ident_d[:, 0:64],
                    widx_t[:, 0:8],
                    128,
                    128,
                    64,
                    elem_step=P,
                    single_packet=False,
                    queue_num=q,
                )

            def row_dma(loc_a, loc_b, src_tile, b, wt):
                """DMA rows [b*P, b*P+wt) of the local table to the A/B
                halves (split at local index ha; ha is block-aligned)."""
                r0 = b * P
                if r0 < ha:
                    nc.sync.dma_start(
                        out=loc_a[r0 : r0 + wt, :], in_=src_tile[:wt, :]
                    )
                else:
                    nc.sync.dma_start(
                        out=loc_b[r0 - ha : r0 - ha + wt, :], in_=src_tile[:wt, :]
                    )

            # ---- layer-1 GEMM: hs = dinv * (x @ W0), kept in SBUF + DRAM ----
            # zero the padded tail of the last block first: its rows feed the
            # self-loop identity matmul, and NaN garbage * 0 = NaN on the PE
            nc.vector.memset(hs_all[:, nblk - 1, :], 0.0)
            for t in range(nblk):
                wt = min(P, npc - t * P)
                hp = pgemm.tile([P, f1], F32, name="hp")
                nc.tensor.matmul(
                    hp[:wt, :],
                    xT_t[:, t * P : t * P + wt],
                    w0_t[:],
                    start=True,
                    stop=True,
                )
                nc.scalar.activation(
                    hs_all[:wt, t, :],
                    hp[:wt, :],
                    AF.Copy,
                    scale=dinv_t[:wt, t : t + 1],
                )
                row_dma(locs[0][0], locs[0][1], hs_all[:, t, :], t, wt)
                if t == blk_a - 1:
                    allgather(locs[0][0], tabs[0][0])
            allgather(locs[0][1], tabs[0][1])

            # big constant loads deferred here so the GEMM row-DMAs (which
            # gate AG-L1A) are not queued behind them on the sync DMA queue;
            # these finish during the AllGathers, before the first consumers
            idx_t = load_const("idx_t", idx_d, [P, plan.tot16], I16)
            slots_t = load_const("slots_t", slots_d, [P, plan.ncols])
            w1_t = load_const("w1_t", w1_d, [f1, f2], BF16)
            b0_t = load_const("b0_t", b0_d, [1, f1])
            b1_t = load_const("b1_t", b1_d, [1, f2])
            iota_t = load_const("iota_t", iota_d, [P, P])
            identb_t = load_const("identb_t", identb_d, [P, P], BF16)
            sqrow_t = load_const("sqrow_t", sqrow_d, [1, nblk * P])

            qc = [0]  # global gather-queue cycle

            MAXC = 16  # max 128-row columns per dma_gather (smaller gathers
            # pipeline better through the gpsimd dispatch overlap)

            def gather_half(layer, gi, h):
                nid = plan.gather_nid[(gi, h)]
                if nid == 0:
                    return None
                ncol = (nid + P - 1) // P
                i0 = plan.seg_idx16[(gi, h)]
                tiles = []
                for j in range(0, ncol, MAXC):
                    nch = min(MAXC, ncol - j)
                    nid_j = nch * P
                    g_tile = gpool.tile(
                        [P, nch, f1], BF16, tag="gath", name=f"g{layer}{gi}_{h}_{j}"
                    )
                    ij = i0 + j * P // 16
                    nc.gpsimd.dma_gather(
                        g_tile[:],
                        tabs[layer - 1][h][:, :],
                        idx_t[:, ij : ij + nid_j // 16],
                        nid_j,
                        nid_j,
                        f1,
                        single_packet=False,
                        queue_num=qc[0] % 4,
                    )
                    qc[0] += 1
                    tiles.append(g_tile)
                return tiles

            def block_pieces(b, h):
                sz = int(plan.SZ[b, h])
                if sz == 0:
                    return None
                off = plan.seg_off[(b, h)]
                return (off // P, sz // P)  # (first col, ncols)

            def build_sel_group(gi, h):
                nid = plan.gather_nid[(gi, h)]
                if nid == 0:
                    return None
                nch = (nid + P - 1) // P
                colb = plan.seg_colbase[(gi, h)]
                sel = spool.tile([P, nch, P], BF16, tag="sel", name="sel")
                nc.vector.tensor_tensor(
                    out=sel[:],
                    in0=slots_t[:, colb : colb + nch].to_broadcast([P, nch, P]),
                    in1=iota_t[:, :]
                    .rearrange("p (a b) -> p a b", a=1)
                    .to_broadcast([P, nch, P]),
                    op=mybir.AluOpType.is_equal,
                )
                return sel

            def seg_matmuls(pb, g_tiles, sel, span, swapped, start, stop):
                c0, nch = span
                for k, c in enumerate(range(c0, c0 + nch)):
                    st = start and k == 0
                    sp = stop and k == nch - 1
                    gt = g_tiles[c // MAXC][:, c % MAXC, :]
                    if not swapped:
                        nc.tensor.matmul(
                            pb[:], sel[:, c, :], gt, start=st, stop=sp,
                        )
                    else:
                        nc.tensor.matmul(
                            pb[:], gt, sel[:, c, :], start=st, stop=sp,
                        )

            def scatter_pass1(layer):
                """Half-A gathers; park each block's partial sum in SBUF."""
                swapped = layer == 2
                for gi, blocks in enumerate(plan.groups):
                    g_tile = gather_half(layer, gi, 0)
                    if layer == 2 and gi == 12:
                        # trigger the half-B AllGather mid-pass: its t2 data
                        # is complete (layer-1 finished), so it never stalls
                        # the gather queue here, and it completes before the
                        # pass-2 gathers reach the head of the queue
                        allgather(locs[1][1], tabs[1][1])
                    sel = build_sel_group(gi, 0)
                    for b in blocks:
                        span = block_pieces(b, 0)
                        if span is None:
                            continue
                        pb = pscat.tile([P, f1], F32, name="pb")
                        seg_matmuls(
                            pb, g_tile, sel, span, swapped,
                            start=True, stop=True,
                        )
                        nc.scalar.activation(part_all[:, b, :], pb[:], AF.Copy)

            def scatter_pass2(layer):
                """Half-B gathers; bias + self-loop + partial re-injection +
                half-B pieces, then close out the block."""
                swapped = layer == 2
                # for layer 1: trigger the next layer's half-A AllGather a
                # couple of groups after block blk_a-1's t2 DMA is issued, so
                # the collective's wait never stalls pending gathers
                ag_group = plan.g_of[blk_a - 1] + 1 if layer == 1 else None
                for gi, blocks in enumerate(plan.groups):
                    g_tile = gather_half(layer, gi, 1)
                    sel = build_sel_group(gi, 1)
                    if gi == ag_group:
                        allgather(locs[1][0], tabs[1][0])
                    for b in blocks:
                        wb = min(P, npc - b * P)
                        span = block_pieces(b, 1)
                        pb = pscat.tile([P, f1], F32, name="pb")
                        if not swapped:
                            nc.tensor.matmul(
                                pb[:],
                                sqrow_t[0:1, b * P : (b + 1) * P],
                                b0_t[:],
                                start=True,
                                stop=False,
                            )
                            nc.tensor.matmul(
                                pb[:], identb_t[:], hs_all[:, b, :],
                                start=False, stop=False,
                            )
                        else:
                            nc.tensor.matmul(
                                pb[:], t2_all[:, b, :], identb_t[:],
                                start=True, stop=False,
                            )
                        if block_pieces(b, 0) is not None:
                            # re-inject the pass-1 partial
                            nc.tensor.matmul(
                                pb[:], identb_t[:], part_all[:, b, :],
                                start=False, stop=span is None,
                            )
                        if span is not None:
                            seg_matmuls(
                                pb, g_tile, sel, span, swapped,
                                start=False, stop=True,
                            )
                        if not swapped:
                            # t2 = dinv * sigmoid(dinv * agg)
                            ob = stpool.tile([P, f1], F32, tag="ob", name="ob")
                            nc.scalar.activation(
                                ob[:], pb[:], AF.Sigmoid,
                                scale=dinv_t[:, b : b + 1],
                            )
                            nc.scalar.activation(
                                t2_all[:, b, :], ob[:], AF.Copy,
                                scale=dinv_t[:, b : b + 1],
                            )
                            row_dma(locs[1][0], locs[1][1], t2_all[:, b, :], b, wb)
                        else:
                            aggT = stpool.tile(
                                [P, f1], BF16, tag="aggT", name="aggT"
                            )
                            nc.scalar.activation(aggT[:], pb[:], AF.Copy)
                            yp = pgemm.tile([P, f2], F32, name="yp")
                            nc.tensor.matmul(
                                yp[:wb, :], aggT[:, :wb], w1_t[:],
                                start=True, stop=False,
                            )
                            nc.tensor.matmul(
                                yp[:wb, :],
                                sqrow_t[0:1, b * P : b * P + wb],
                                b1_t[:],
                                start=False,
                                stop=True,
                            )
                            yo = stpool.tile([P, f2], F32, tag="yo", name="yo")
                            nc.scalar.activation(
                                yo[:wb, :], yp[:wb, :], AF.Copy,
                                scale=dinv_t[:wb, b : b + 1],
                            )
                            nc.sync.dma_start(
                                out=y_d[b * P : b * P + wb, :], in_=yo[:wb, :]
                            )

            scatter_pass1(1)
            scatter_pass2(1)
            scatter_pass1(2)
            scatter_pass2(2)

    nc.compile()
    return nc


def _make_in_maps(x, W0, b0, W1, b1, plan, per_core):
    npc = plan.npc
    x = np.asarray(x, dtype=np.float32)
    shared = dict(
        W0=np.asarray(W0, np.float32).astype(BF),
        W1=np.asarray(W1, np.float32).astype(BF),
        b0=np.asarray(b0, np.float32).reshape(1, -1),
        b1=np.asarray(b1, np.float32).reshape(1, -1),
        iota=np.tile(np.arange(P, dtype=np.float32)[None, :], (P, 1)).copy(),
        ident=np.eye(P, dtype=np.float32),
        identb=np.eye(P, dtype=np.float32).astype(BF),
    )
    in_maps = []
    for c in range(plan.n_cores):
        m = dict(shared)
        m["xT"] = np.ascontiguousarray(x[c * npc : (c + 1) * npc, :].T).astype(BF)
        m["idx16"] = per_core[c]["idx16"]
        m["widx"] = np.zeros((P, 8), np.int16)
        m["slots"] = per_core[c]["slots"]
        m["degt"] = per_core[c]["degt"]
        m["sqrow"] = per_core[c]["sqrow"]
        in_maps.append(m)
    return in_maps


_CACHE = {}


def build(x, edges, W0, b0, W1, b1, n_nodes=N_NODES, n_cores=N_CORES,
          gb=GROUP_BLOCKS):
    """Returns (nc, in_maps, plan). Cached on the edge structure size."""
    plan, per_core = _build_metadata(edges, n_nodes, n_cores, gb)
    key = (n_nodes, n_cores, gb, tuple(plan.SZ.reshape(-1).tolist()))
    if key not in _CACHE:
        _CACHE[key] = _build_nc(plan, x.shape[1], W0.shape[1], W1.shape[1])
    nc = _CACHE[key]
    in_maps = _make_in_maps(x, W0, b0, W1, b1, plan, per_core)
    return nc, in_maps, plan


def kernel(x, edges, W0, b0, W1, b1):
    x = np.asarray(x)
    nc, in_maps, plan = build(x, edges, W0, b0, W1, b1)
    res = run_bass_kernel_spmd(nc, in_maps, list(range(plan.n_cores)))
    y = np.concatenate([r["y"] for r in res.results], axis=0)
    return y.astype(np.float32)
